# revision 59
# baseline (speedup 1.0000x reference)
"""ConvLSTM (pixel-wise, 1x1 convs) Trainium2 Bass kernel.

Math (after exact algebraic folding):
  per pixel, per t:  g1 = W1x @ x_t + W1h @ h1 + b1   (W1x = Wih1 @ (W_red * denorm_scale))
                     i,f,g,o = split(g1); c1 = sig(f)*c1 + sig(i)*tanh(g); h1 = sig(o)*tanh(c1)
                     g2 = W21 @ h1 + W22 @ h2 + b2    (W21 = Wih2 @ Wc1)
                     c2,h2 analogous
  out = (W_head @ Wc2) @ h2_final + const

Sharding: batch b -> core b (8 cores, no collectives).

Per-core layout (one chunk = all 16384 pixels):
  S1[0..1] [128, HW] bf16 (alternating by t): rows 0:64 h1(t-1),
      rows 64:92 x(t), row 92 ones (shipped as a 29th x channel).
      One K=93 matmul per gate computes W1h@h1 + W1x@x + b1. The x(t+1)
      frame DMA is issued at slot (t,0); since cell2 no longer touches
      these tiles (see V), the only WAR is against cell1 reads of t-1,
      so the ~36us 29-partition-bound transfer hides fully.
  V [128, HW] bf16: rows 0:64 h1(t) (same data as S1next rows 0:64,
      placed twice), rows 64:128 h2. Cell2 gates are ONE K=128 matmul
      per gate-half -- W21@h1 + W22@h2 fused via stacked weights
      (w2 = [W21.T; W22.T]), halving cell2 PE time vs a two-mm
      accumulation scheme (PE streams N cols per mm regardless of K).
  c1/c2 [128, ngrp, fd] bf16: A-half pixels on partitions 0:64, B-half
      on 64:128.
  Gates land in ONE [128, 4, fd] PSUM tile per cell in order (i,f,o,g);
  ONE Tanh covers all four gates: sigma(x) = (tanh(x/2)+1)/2 with the
  1/2 folded into gate weights and doubled c/h folded into consumer
  weights (bf16-safe: only near-zero tanh values are ever stored).
  The gate ACT writes a STRIDED output sfo[128, 4, 2, fd] (gate-major,
  slot-parity minor) so that all downstream pointwise work runs on
  PAIRED [128, 2*fd] tiles covering two slots: this amortizes the
  ~150-cycle fixed cost of every DVE instruction (the pointwise STTs
  have no 2x perf mode -- scalar_tensor_tensor is 1x-only) and halves
  the DVE queue depth. h placement into S1next rides the otherwise-idle
  SP HW-DGE ring (read a timestep later; x frames use the Act ring so
  a 36us frame never queues ahead of a placement copy).
  Emission is software-pipelined: cell2 lags cell1 by lag=2 pairs and
  its tanh(c)+h-placement trail one iteration further, so the in-order
  Scalar queue runs [act1,act1,act2,act2,tc1,tc2] with every input
  ready on arrival (the lag is a sharp optimum: lag 1 or 3 costs
  +140us). h placement into S1next and h2 into V ride the SP HW-DGE
  queues (read a timestep later; x frames use the Act ring so a 36us
  frame never queues ahead of a placement copy; startup frames trigger
  from the Sync queue so ring backpressure cannot block the first
  ACTIVATEs). Deep hp pool (6 bufs) decouples the copy-DMA read
  latency from the producing STT.
  Engine balance per slot-pair (~10.8us): DVE 8 STT [128,1024] + 2 V
  copies ~9.7us (88%), ACT 4 gate-tanh + 2 paired tanh_c ~9.7us (88%),
  PE 16 col-tiled mm pairs ~7.1us (PE is HAM-cold at 1.2 GHz
  permanently in this environment; col-tiled 64-row pairs stream
  concurrently). GPSIMD is compute-poison: ~2us per op AND it
  throttles concurrent 2-port DVE ops ~2.5x via the shared 2nd SBUF
  port (measured) -- only the one-time V memset goes there. Weights
  ship pre-cast to bf16; the head's LDWEIGHTS-bound chunk matmuls
  overlap the flush pairs on the otherwise-idle PE.
  Measured: 983us (session-start baseline) -> 692us; rel err 8.0e-3.
"""

import numpy as np
import ml_dtypes

import concourse.bass as bass
import concourse.tile as tile
from concourse import bacc, mybir
from concourse.bass_utils import run_bass_kernel_spmd

F32 = mybir.dt.float32
BF16 = mybir.dt.bfloat16
AF = mybir.ActivationFunctionType

T, CIN, HID = 8, 28, 64
H = W = 128
HW = H * W            # pixels per core (one batch element)
HALF = HW // 2
NCORES = 8
K1 = HID + CIN + 1    # 93: h1 rows, x rows, ones row

import os
CFG = dict(
    fd=512,            # pixels per half per group (psum: 2 cells x [128, 4, fd] fp32)
    s1_depth=2,        # S1 tile rotation depth
    plane_bufs=2,
    dmacopy=1,
    lag=2,           # cell2 emission lag in pairs
    dmaslot=0,       # group index within a timestep to issue the next x frame
    xring=0,         # 1: in-loop x frames on the Act ring, 0: Sync ring
    xsplit=4,        # in-loop frame DMA split (parallel queue engines)         # h1->S1next placement via SBUF->SBUF DMA on the SP ring
)
for _k in list(CFG):
    _v = os.environ.get(f"KCFG_{_k.upper()}")
    if _v is not None:
        CFG[_k] = int(_v) if _v.isdigit() else _v


def _fold_weights(inputs):
    """Host-side exact algebraic folding (all fp32 numpy)."""
    f = np.float32
    W_red = inputs["W_red"].astype(f)
    b_red = inputs["b_red"].astype(f)
    # de-normalization of channels 11 (u) and 12 (v), folded into W_red
    a = np.ones(CIN, f); a[11] = f(0.15); a[12] = f(0.12)
    d = np.zeros(CIN, f); d[11] = f(0.02); d[12] = f(-0.01)
    W_red_eff = W_red * a[None, :]
    b_red_eff = b_red + W_red @ d

    W1x = inputs["Wih1"].astype(f) @ W_red_eff          # [256, 28]
    W1h = inputs["Whh1"].astype(f)                      # [256, 64]
    b1 = (inputs["bih1"] + inputs["bhh1"]).astype(f) + inputs["Wih1"].astype(f) @ b_red_eff
    W21 = inputs["Wih2"].astype(f) @ inputs["Wc1"].astype(f)   # [256, 64]
    W22 = inputs["Whh2"].astype(f)                      # [256, 64]
    b2 = (inputs["bih2"] + inputs["bhh2"]).astype(f) + inputs["Wih2"].astype(f) @ inputs["bc1"].astype(f)
    whead = (inputs["W_head"].astype(f) @ inputs["Wc2"].astype(f))[0]     # [64]
    bhead = float((inputs["W_head"].astype(f) @ inputs["bc2"].astype(f) + inputs["b_head"].astype(f)).reshape(()))

    # reorder gate blocks (i, f, g, o) -> (i, f, o, g) so the three
    # sigmoid gates are contiguous in the PSUM tile
    perm = np.r_[0:64, 64:128, 192:256, 128:192]
    W1x, W1h, W21, W22 = W1x[perm], W1h[perm], W21[perm], W22[perm]
    b1, b2 = b1[perm], b2[perm]

    w1 = np.zeros((128, 256), f)
    w1[0:HID] = W1h.T
    w1[HID:HID + CIN] = W1x.T
    w1[HID + CIN] = b1
    # combined cell2 weight: rows 0:64 read h1 (V rows 0:64), rows
    # 64:128 read h2 (V rows 64:128) -- one K=128 matmul per gate-half
    w2 = np.zeros((128, 256), f)
    w2[0:HID] = W21.T
    w2[HID:128] = W22.T
    # cell2 bias rides an extra K=29 matmul against S1next's [x; ones]
    # rows only when nonzero (it is zero for the reference weights)
    w2c = None
    if np.any(b2 != 0):
        w2c = np.zeros((128, 256), f)
        w2c[HID + CIN] = b2
    wh = np.zeros((128, 1), f)
    wh[HID:, 0] = whead
    d = dict(w1=w1, w2=w2, wh=wh)
    if w2c is not None:
        d["w2c"] = w2c
    # vtanh folds: sigma(x) = (tanh(x/2) + 1)/2 -- halve the i,f,o gate
    # pre-activations (cols 0:192) so ONE Tanh covers all four gates; h is
    # stored doubled (h_hat = (tanh(o-pre)+1)*tanh(c)) so halve every weight
    # row that reads it; c is stored doubled too (tanh(c) uses scale=0.5).
    for nm, M in d.items():
        if nm != "wh":
            M[:, 0:192] *= 0.5
    d["w1"][0:HID] *= 0.5     # rows reading h1_hat
    d["w2"][0:HID] *= 0.5     # rows reading h1_hat
    d["w2"][HID:128] *= 0.5   # rows reading h2_hat
    d["wh"] = wh * 0.5
    return d, bhead


def build(nc, bhead, has_b2):
    fd = CFG["fd"]
    ngrp = HALF // fd
    NSLOT = T * ngrp
    ND = CFG["s1_depth"]
    fd2 = 2 * fd
    Alu = mybir.AluOpType

    x_d = nc.dram_tensor("xt", [T, CIN + 1, HW], BF16, kind="ExternalInput").ap()
    w_names = ["w1", "w2", "wh"] + (["w2c"] if has_b2 else [])
    w_dram = {nm: nc.dram_tensor(nm, [128, 1] if nm == "wh" else [128, 256], BF16,
                                 kind="ExternalInput").ap() for nm in w_names}
    # out[i, j] = pixel j*128 + i of this core's [H, W] map (host transposes)
    out_d = nc.dram_tensor("out", [128, HW // 128], F32, kind="ExternalOutput").ap()

    with tile.TileContext(nc) as tc:
        with (
            tc.tile_pool(name="const", bufs=1) as const,
            tc.tile_pool(name="state", bufs=1) as state,
            tc.tile_pool(name="planes", bufs=CFG["plane_bufs"]) as planes,
            tc.tile_pool(name="hplanes", bufs=7) as hplanes,
            tc.tile_pool(name="pq", bufs=1) as pq,
            tc.tile_pool(name="outp", bufs=1) as outp,
            tc.tile_pool(name="psum", bufs=1, space=bass.MemorySpace.PSUM) as psum,
        ):
            # Weights arrive pre-cast to bf16 from the host: no staging
            # tiles, no DVE converts ahead of the first STTs.
            w_sb = {}
            for nm in w_names:
                shp = [128, 1] if nm == "wh" else [128, 256]
                wb = const.tile(shp, BF16, tag=nm, name="wb")
                nc.sync.dma_start(wb[:], w_dram[nm])
                w_sb[nm] = wb

            Ss = [state.tile([128, HW], BF16, tag=f"S1_{i}", name=f"S1_{i}")
                  for i in range(ND)]
            V = state.tile([128, HW], BF16, tag="V")
            c1 = state.tile([128, ngrp, fd], BF16, tag="c1")
            c2 = state.tile([128, ngrp, fd], BF16, tag="c2")
            out_sb = outp.tile([128, HW // 128], F32, tag="osb")

            # h2(-1) = 0 for cell2's first K=128 matmul. GPSIMD is slow but
            # this is one-time and fully hidden behind the first x-frame DMA;
            # it keeps the 13.7us DVE memset off the warm-up critical path.
            nc.gpsimd.memset(V[HID:128, :], 0.0)

            # x frames ride the Activation HW-DGE ring (one ~700ns trigger
            # per timestep on the Scalar queue) so the SP ring stays free for
            # the h1 placement copies -- the rings are in-order, and a 36us
            # frame transfer queued ahead of a copy stalls the pipeline.
            xdma = (nc.scalar if CFG["xring"] else nc.sync) if CFG["dmacopy"] else nc.sync

            def dma_frame(f, split):
                dst = Ss[f % ND]
                if split:
                    # Startup frames: chunks in group-need order (A0,B0,A1,..)
                    # so the first slot's columns land early, triggered from
                    # the SYNC queue -- a ring-backpressured trigger on the
                    # Scalar queue would block the first ACTIVATEs (~30us).
                    CW = HW // 8
                    for i in range(4):
                        for half in (0, HALF):
                            c0 = half + i * CW
                            nc.sync.dma_start(dst[HID:K1, c0:c0 + CW],
                                              x_d[f][:, c0:c0 + CW])
                else:
                    # separate dma_starts land on separate queue engines and
                    # run in parallel (a single dma_start's descriptor chain
                    # serializes on one engine at ~27 GB/s)
                    QW = HW // CFG["xsplit"]
                    for q in range(CFG["xsplit"]):
                        xdma.dma_start(dst[HID:K1, q * QW:(q + 1) * QW],
                                       x_d[f][:, q * QW:(q + 1) * QW])

            dma_frame(0, split=True)
            dma_frame(1, split=True)

            # per-slot / per-pair live tile handles
            P0s, P1s, pl1, pl2 = {}, {}, {}, {}

            def slot_tg(s):
                return s // ngrp, s % ngrp

            def cols(g):
                return g * fd, HALF + g * fd      # A-half / B-half col starts

            def c1_mms(s):
                t, g = slot_tg(s)
                Scur = Ss[t % ND]
                if g == CFG["dmaslot"] and t + 1 < T and t > 0:
                    dma_frame(t + 1, split=False)
                a0, b0 = cols(g)
                ks = slice(0, K1) if t > 0 else slice(HID, K1)
                P = psum.tile([128, 4, fd], F32, tag="P0", name="P0")
                P0s[s] = P
                for q in range(4):
                    for (cb, po) in ((a0, 0), (b0, 64)):
                        nc.tensor.matmul(
                            P[po:po + 64, q, :],
                            w_sb["w1"][ks, q * 64:(q + 1) * 64],
                            Scur[ks, cb:cb + fd],
                        )

            def c2_mms(s):
                t, g = slot_tg(s)
                a0, b0 = cols(g)
                P = psum.tile([128, 4, fd], F32, tag="P1", name="P1")
                P1s[s] = P
                # ONE K=128 mm per gate-half: V rows 0:64 = h1(t) (written
                # by this pair's h placement), rows 64:128 = h2(t-1).
                # Alternating column halves -> col-tiled pairs run
                # concurrently on the PE's two 64-col groups.
                halves = ((a0, 0), (b0, 64))
                for wave in range(2):
                    for q in range(4):
                        cb, po = halves[(q + wave) % 2]
                        nc.tensor.matmul(
                            P[po:po + 64, q, :],
                            w_sb["w2"][0:128, q * 64:(q + 1) * 64],
                            V[0:128, cb:cb + fd],
                            start=True, stop=not has_b2,
                        )
                        if has_b2:
                            Snxt = Ss[(t + 1) % ND]
                            nc.tensor.matmul(
                                P[po:po + 64, q, :],
                                w_sb["w2c"][HID:K1, q * 64:(q + 1) * 64],
                                Snxt[HID:K1, cb:cb + fd],
                                start=False, stop=True,
                            )

            def act_gates(s, Ps, sfo, pl):
                # strided write: gate q of slot-parity e lands at
                # sfo[:, q, e, :] -> every gate's slot-PAIR is a contiguous
                # [128, 2*fd] block for the paired pointwise stage.
                e = s % 2
                P = Ps.pop(s)
                nc.scalar.activation(sfo[:, :, e, :], P[:, :, :], AF.Tanh)
                pl[s] = sfo

            def dve_c_pair(s0, pl, cc, tagp):
                # paired across slots (s0, s0+1), all tiles [128, 2*fd]:
                #   c_hat = (v_f+1)*c_hat*0.5 + (v_i+1)*tanh(g)
                t, g = slot_tg(s0)
                sfo = pl[s0]
                si = sfo[:, 0, :, :]
                sf = sfo[:, 1, :, :]
                tg = sfo[:, 3, :, :]
                cg = cc[:, g:g + 2, :]
                if t > 0:
                    # bufs=1 is free: the next pair's writes follow this
                    # pair's reads in the same in-order DVE queue anyway
                    p = pq.tile([128, fd2], BF16, tag=f"t2{tagp}")
                    q = pq.tile([128, fd2], BF16, tag=f"t1{tagp}")
                    nc.vector.scalar_tensor_tensor(
                        p[:], si, 1.0, tg, Alu.add, Alu.mult)
                    nc.vector.scalar_tensor_tensor(
                        q[:], sf, 1.0, cg, Alu.add, Alu.mult)
                    # split the 1x STT combine into 4x TS (in-place halve)
                    # + 2x bf16 TT add: ~1017ns vs ~1224ns at pair width
                    nc.vector.tensor_scalar_mul(q[:], q[:], 0.5)
                    nc.vector.tensor_add(cg, q[:], p[:])
                else:
                    nc.vector.scalar_tensor_tensor(
                        cg, si, 1.0, tg, Alu.add, Alu.mult)

            def act_tc_pair(s0, pl, cc, tagp):
                t, g = slot_tg(s0)
                tch = planes.tile([128, fd2], BF16, tag=f"tch{tagp}")
                nc.scalar.activation(tch[:], cc[:, g:g + 2, :], AF.Tanh, scale=0.5)
                pl[s0] = (pl[s0], tch)

            def h_muls_pair(s0, pl, dsts, tagp):
                # h_hat = (v_o+1)*tanh(c) = 2h, built once in a [128, 2*fd]
                # plane, then per-half bf16 copies into the state rows.
                # dsts: list of (tile, row_slice, via_dma).
                t, g = slot_tg(s0)
                sfo, tch = pl.pop(s0)
                a0, b0 = cols(g)
                so = sfo[:, 2, :, :]
                # deep pool: the S1next placement DMA reads hp with multi-us
                # latency; 3 bufs keep the STT from stalling on the WAR
                hp = hplanes.tile([128, fd2], BF16, tag=f"hp{tagp}")
                nc.vector.scalar_tensor_tensor(
                    hp[:], so, 1.0, tch[:], Alu.add, Alu.mult)
                for dst, rows, via_dma in dsts:
                    eng = nc.sync if via_dma else nc.vector
                    if via_dma:
                        eng.dma_start(dst[rows, a0:a0 + fd2], hp[0:64, :])
                        eng.dma_start(dst[rows, b0:b0 + fd2], hp[64:128, :])
                    else:
                        eng.tensor_copy(dst[rows, a0:a0 + fd2], hp[0:64, :])
                        eng.tensor_copy(dst[rows, b0:b0 + fd2], hp[64:128, :])

            sfo1 = lambda: planes.tile([128, 4, 2, fd], BF16, tag="sfo1",
                                       name="sfo1")
            sfo2 = lambda: planes.tile([128, 4, 2, fd], BF16, tag="sfo2",
                                       name="sfo2")

            # Cell2 lags ONE PAIR: the per-engine queues execute in emission
            # order, and this order keeps every gate-ACT's inputs ready by
            # the time the Scalar queue reaches it -- the 4 big gate tanh
            # ops stream back-to-back while the DVE chain of the current
            # pair and the tc/h tail of the previous pair fill the gaps.
            def cell1_pair(s0):
                t, g = slot_tg(s0)
                f1 = sfo1()
                c1_mms(s0)
                act_gates(s0, P0s, f1, pl1)
                c1_mms(s0 + 1)
                act_gates(s0 + 1, P0s, f1, pl1)
                del pl1[s0 + 1]

            def cell2_pair_front(s0):
                f2 = sfo2()
                c2_mms(s0)
                act_gates(s0, P1s, f2, pl2)
                c2_mms(s0 + 1)
                act_gates(s0 + 1, P1s, f2, pl2)
                del pl2[s0 + 1]

            def cell1_pair_back(s0):
                t, g = slot_tg(s0)
                S1next = Ss[(t + 1) % ND]
                dve_c_pair(s0, pl1, c1, "a")
                act_tc_pair(s0, pl1, c1, "a")
                # V placement first so cell2's matmuls release earliest;
                # S1next placement is dead at the last timestep and is only
                # read one timestep later, so it can ride the idle DMA ring.
                d1 = [(V, slice(0, HID), False)]
                if t + 1 < T:
                    d1.append((S1next, slice(0, HID), bool(CFG["dmacopy"])))
                h_muls_pair(s0, pl1, d1, 'a')

            def cell2_pair_dve(s0):
                dve_c_pair(s0, pl2, c2, "b")

            def cell2_pair_tail(s0):
                act_tc_pair(s0, pl2, c2, "b")
                # h2 is read one timestep later -> DMA queues, like S1next
                h_muls_pair(s0, pl2, [(V, slice(HID, 128), bool(CFG["dmacopy"]))],
                            'b')

            # cell2's tanh(c)+h placement run one iteration later still:
            # their DVE input (c2' of pair k-LAG) only completes near the end
            # of iteration k, so emitting them at k would stall the in-order
            # Scalar queue right where the next pair's gate-acts should run.
            LAG = 2 * CFG["lag"]
            for s0 in range(0, NSLOT, 2):
                cell1_pair(s0)
                if s0 >= LAG:
                    cell2_pair_front(s0 - LAG)
                cell1_pair_back(s0)
                if s0 >= LAG + 2:
                    cell2_pair_tail(s0 - LAG - 2)
                if s0 >= LAG:
                    cell2_pair_dve(s0 - LAG)
            # head: out[pix] = whead @ h2[pix] + bhead, pixels as matmul
            # M-dim. P0 is dead after the main loop, so the ~13.7us of
            # LDWEIGHTS-bound head chunks for groups whose h2 already landed
            # overlap the flush pairs' ACT/DVE work on the idle PE.
            ncols = HW // 128
            ph = psum.tile([128, ncols], F32, tag="P0", name="ph")

            def head_blocks(jlist):
                for j in jlist:
                    nc.tensor.matmul(
                        ph[:, j:j + 1],
                        V[HID:128, j * 128:(j + 1) * 128],
                        w_sb["wh"][HID:128, 0:1],
                    )

            # last in-loop tail covers pair (NSLOT-2)-LAG-2 -> its 2 groups
            glast = (NSLOT - 2 - LAG - 2) % ngrp + 2  # first group NOT tailed in-loop
            early = [j for j in range(ncols) if (j % 64) // 4 < glast]
            late = [j for j in range(ncols) if (j % 64) // 4 >= glast]
            head_blocks(early)
            for s0 in range(NSLOT - LAG, NSLOT, 2):
                cell2_pair_front(s0)
                cell2_pair_dve(s0)
            for s0 in range(NSLOT - LAG - 2, NSLOT, 2):
                cell2_pair_tail(s0)
            head_blocks(late)
            nc.vector.tensor_scalar_add(out_sb[:], ph[:], float(bhead))
            nc.sync.dma_start(out_d, out_sb[:])
    nc.compile()
    return nc


def _make_nc():
    # Bacc (not raw Bass): its compile() runs move_matmul_waits_to_ldweights +
    # generate_event_semaphores, required to satisfy TRN2's 1-wait-per-inst limit.
    return bacc.Bacc("TRN2", target_bir_lowering=False, debug=False,
                     num_devices=NCORES, enable_partition_id=False)


def _in_maps(inputs):
    folded, _ = _fold_weights(inputs)
    folded = {k: v.astype(ml_dtypes.bfloat16) for k, v in folded.items()}
    x = np.asarray(inputs["x"], dtype=np.float32)
    x_bf = x.reshape(NCORES, T, CIN, HW).astype(ml_dtypes.bfloat16)
    ones = np.ones((T, 1, HW), ml_dtypes.bfloat16)
    maps = []
    for b in range(NCORES):
        m = dict(folded)
        m["xt"] = np.ascontiguousarray(
            np.concatenate([x_bf[b], ones], axis=1))
        maps.append(m)
    return maps


def _assemble(results):
    out = np.empty((NCORES, H, W), np.float32)
    for b in range(NCORES):
        o = results[b]["out"]          # [128, HW//128], o[i, j] = pixel j*128+i
        out[b] = o.T.reshape(H, W)
    return out


def _run(inputs, trace=False):
    folded, bhead = _fold_weights(inputs)
    nc = build(_make_nc(), bhead, "w2c" in folded)
    maps = _in_maps(inputs)
    res = run_bass_kernel_spmd(nc, maps, core_ids=list(range(NCORES)), trace=trace)
    return _assemble(res.results), res


def kernel(**inputs) -> np.ndarray:
    out, _ = _run(inputs, trace=False)
    return out
